# revision 36
# baseline (speedup 1.0000x reference)
"""Trainium2 Bass kernel for 2-layer GCN (nn_GCN_39848706573686).

Node-sharded across 8 NeuronCores (12500 nodes/core + pad). Three SPMD
launches:
  L1: g = dis^ * (x @ W1) per-core shard       (TensorE + DVE, bf16)
  L2: conv1 padded-ELL segment reduce + relu/W2 fused   (DVE tree-fold)
  L3: conv2 padded-ELL segment reduce + bias            (DVE tree-fold)
The host performs only integer routing: edge bucketing by destination,
degree counting, ELL slot index construction, and the halo-exchange row
replication between launches. All floating-point arithmetic runs on the
NeuronCores (host computes only the 1/sqrt(deg) normalization table).

Perf notes vs the first working version:
  - segment reduction uses in-place pairwise tree-folds with
    tensor_tensor adds in bf16 (DVE 2x_1p mode, ~0.5ns/elem) instead of
    tensor_reduce (no fast modes, ~1.04ns/elem);
  - all constants ride as prefix columns of the main slot tensor
    (f32 values bitcast into bf16 column pairs) so each launch does a
    single chunked input DMA stream - no small-descriptor const DMAs;
  - L1 packs 8 node-slices x 16 channels into the full 128 partitions
    so the per-node dis scaling costs 4 full-width DVE ops;
  - kernel outputs are bf16 where the consumer is the next launch's
    slot scatter (g, g2), f32 only for the final output.
"""
import os
import sys
import types
import numpy as np

# --- environment bootstrap (self-contained copy of bassboot logic) -----
for _p in ("/opt/trn_rl_repo", "/root/patched"):
    if _p not in sys.path and os.path.isdir(_p):
        sys.path.insert(0, _p)

from concourse import bass, bacc, mybir, tile  # noqa: E402
from concourse import bass_utils  # noqa: E402


def _install_ntff_hook():
    if "antenv.axon_hooks" not in sys.modules:
        mod = types.ModuleType("antenv.axon_hooks")
        _h = {}
        mod.set_axon_ntff_profile_hook = lambda h: _h.__setitem__("h", h)
        mod.get_axon_ntff_profile_hook = lambda: _h.get("h")
        sys.modules["antenv.axon_hooks"] = mod
        try:
            import antenv
            antenv.axon_hooks = mod
        except ImportError:
            pass
    mod = sys.modules["antenv.axon_hooks"]
    if mod.get_axon_ntff_profile_hook() is None:
        try:
            from trn_agent_boot.trn_boot import _ntff_profile_via_ctypes
            hook = _ntff_profile_via_ctypes("/opt/axon/libaxon_pjrt.so")
            if hook is not None:
                mod.set_axon_ntff_profile_hook(hook)
        except Exception:
            pass
    bass_utils.upload_artifacts = lambda tmpdir: str(tmpdir)


_install_ntff_hook()

# --- problem constants -------------------------------------------------
N, E, F, H = 100000, 3200000, 128, 16
NC = 8
SH = 12500                  # real nodes per core
SHP = 12544                 # padded rows per core (= 98 * 128)
NB = 98                     # node blocks of 128 per core
SLC = SHP // 4              # 3136 nodes per L1 slice (one per PE quadrant)

# L2/L3 const-prefix column counts (bf16 columns)
C2_DIS2 = 0                  # [0,196) dis^2 f32 (196 bf16 cols)
C2_DIS = 196                 # [196,392) dis f32
C2_B1 = 392                  # [392,424) b1 f32 (16 f32 = 32 cols)
C2_W2 = 424                  # [424,440) w2 bf16
C2_EYE = 440                 # [440,568) 128x128 identity bf16 (PE reduce)
CONST2 = 568
C3_DIS = 0                   # [0,196) dis f32
C3_B2 = 196                  # [196,198) b2 f32
CONST3 = 198

CHUNK2 = 6912                # slot columns per L2 DMA chunk
CHUNK3 = 2048                # slot columns per L3 DMA chunk
MAXNBSEG = 32                # max blocks per segment (PSUM bank = 512 f32)

FT = mybir.dt.float32
BF = mybir.dt.bfloat16
ADD = mybir.AluOpType.add
MULT = mybir.AluOpType.mult

_cached = {}

# Track total device time across launches for test harness
last_exec_ns = {}


# ----------------------------------------------------------------------
# L1: g[32q+c, j] = dis[q*3136+j] * sum_f W1[f,c] * x[q*3136+j, f]
# (4 node slices live in the 4 PE column quadrants; the upper 16
#  partitions of each quadrant are unused garbage)
# ----------------------------------------------------------------------
def _build_l1():
    P0 = 16 + SLC            # w1 cols + dis_sl cols
    NXP = 4                  # xT DMA pieces == node slices
    NJ = 8
    JW = SLC // NJ           # 392 cols per psum piece (one PSUM bank)
    nc = bacc.Bacc("TRN2", target_bir_lowering=False, debug=False,
                   num_devices=NC)
    xin = nc.dram_tensor("xin", [128, P0 + SHP], BF,
                         kind="ExternalInput").ap()
    g = nc.dram_tensor("g", [128, SLC], BF, kind="ExternalOutput").ap()
    with tile.TileContext(nc) as tc:
        with tc.tile_pool(name="sb", bufs=NXP) as sb, \
             tc.tile_pool(name="cst", bufs=1) as cst, \
             tc.tile_pool(name="ps", bufs=1, space="PSUM") as ps:
            p0 = cst.tile([128, P0], BF)
            nc.sync.dma_start(out=p0[:], in_=xin[:, :P0])
            pieces = []
            for pc in range(NXP):
                xt_p = sb.tile([128, SLC], BF, tag="xtp")
                nc.sync.dma_start(
                    out=xt_p[:], in_=xin[:, P0 + pc * SLC:P0 + (pc + 1) * SLC])
                pieces.append(xt_p)
            g_t = cst.tile([128, SLC], BF)
            psj = []
            for j in range(NJ):
                ps_t = ps.tile([128, JW], FT, space="PSUM", tag=f"ps{j}")
                psj.append(ps_t)
            w1_ap = p0[:, 0:16]
            for s in range(4):
                for j in range(NJ):
                    nc.tensor.matmul(
                        out=psj[j][32 * s:32 * s + 16, :],
                        lhsT=w1_ap,
                        rhs=pieces[s][:, j * JW:(j + 1) * JW],
                        start=True, stop=True,
                        tile_position=(0, 32 * s))
            for j in range(NJ):
                nc.vector.tensor_tensor(
                    out=g_t[:, j * JW:(j + 1) * JW], in0=psj[j][:],
                    in1=p0[:, 16 + j * JW:16 + (j + 1) * JW], op=MULT)
            nc.sync.dma_start(out=g[:], in_=g_t[:])
    nc.compile()
    return nc


# ----------------------------------------------------------------------
# chunk planning: split the 98 blocks into DMA chunks, group equal caps
# ----------------------------------------------------------------------
def _plan_chunks(caps, d, prefix, chunk_cols):
    """Returns (total_cols, chunks); chunks = [(col_off, col_len,
    [(cap, b0, nb, soff_in_chunk), ...]), ...]. Chunk 0 includes the
    const prefix."""
    blocks = [d * c for c in caps]
    chunks = []
    cur = []                # list of (b, cap)
    cur_cols = prefix
    col_off = 0
    for b in range(NB):
        c = blocks[b]
        if cur and cur_cols + c > chunk_cols:
            chunks.append((col_off, cur_cols, cur))
            col_off += cur_cols
            cur, cur_cols = [], 0
        cur.append((b, caps[b]))
        cur_cols += c
    if cur:
        chunks.append((col_off, cur_cols, cur))
    # carve a small final chunk so the trailing compute after the last
    # DMA completes quickly
    tail_target = max(chunk_cols // 5, 1200)
    if len(chunks) > 1 and chunks[-1][1] > 2 * tail_target:
        col_off, clen, blist = chunks.pop()
        keep = clen
        cur = list(blist)
        tail = []
        tail_cols = 0
        while cur and tail_cols + d * cur[-1][1] <= tail_target:
            b, cap = cur.pop()
            tail.insert(0, (b, cap))
            tail_cols += d * cap
        if cur and tail:
            chunks.append((col_off, keep - tail_cols, cur))
            chunks.append((col_off + keep - tail_cols, tail_cols, tail))
        else:
            chunks.append((col_off, keep, blist))
    out = []
    for ci, (coff, clen, blist) in enumerate(chunks):
        segs = []
        j = 0
        soff = prefix if ci == 0 else 0
        while j < len(blist):
            b0, cap = blist[j]
            nb = 1
            while (j + nb < len(blist) and blist[j + nb][1] == cap
                   and nb < MAXNBSEG):
                nb += 1
            segs.append((cap, b0, nb, soff))
            soff += nb * d * cap
            j += nb
        out.append((coff, clen, segs))
    total = sum(cl for _, cl, _ in out)
    return total, out


def _emit_folds(nc, v, res_out, scratch=None):
    """Pairwise tree-fold of v [128, w, m] along the MIDDLE axis
    (w-major segment layout: the innermost m = nb*d elements stay a
    contiguous, even-length, 4B-aligned run at every level, keeping the
    DVE 2x_1p fast mode engaged). The final 2->1 add writes
    res_out [128, 1, m]. If `scratch` is given, the first fold level
    writes there instead of in-place, so the source tile is never
    written (lets PE matmuls on other segments of the same tile run
    concurrently)."""
    eng = nc.vector
    w = v.shape[1]
    m = v.shape[2]
    if scratch is not None and w > 2:
        h = w // 2
        sv = scratch[:, :h * m].rearrange("p (w m) -> p w m", w=h, m=m)
        eng.tensor_tensor(out=sv, in0=v[:, 0:h, :],
                          in1=v[:, h:2 * h, :], op=ADD)
        if w % 2:
            eng.tensor_tensor(out=sv[:, 0:1, :], in0=sv[:, 0:1, :],
                              in1=v[:, 2 * h:2 * h + 1, :], op=ADD)
        v, w = sv, h
    while w > 2:
        h = w // 2
        eng.tensor_tensor(out=v[:, 0:h, :], in0=v[:, 0:h, :],
                          in1=v[:, h:2 * h, :], op=ADD)
        if w % 2:
            eng.tensor_tensor(out=v[:, 0:1, :], in0=v[:, 0:1, :],
                              in1=v[:, 2 * h:2 * h + 1, :], op=ADD)
        w = h
    if w == 2:
        eng.tensor_tensor(out=res_out, in0=v[:, 0:1, :],
                          in1=v[:, 1:2, :], op=ADD)
    else:   # cap == 1
        eng.tensor_copy(out=res_out, in_=v[:, 0:1, :])


# ----------------------------------------------------------------------
# L2: conv1 segment reduce + relu + W2, fused per chunk
# ----------------------------------------------------------------------
def _build_l2(caps, fast):
    total, chunks = _plan_chunks(caps, H, CONST2, CHUNK2)
    maxlen = max(cl for _, cl, _ in chunks)
    maxnb = max(sum(s[2] for s in segs) for _, _, segs in chunks)
    nc = bacc.Bacc("TRN2", target_bir_lowering=False, debug=False,
                   num_devices=NC)
    slots = nc.dram_tensor("slots", [128, total], BF,
                           kind="ExternalInput").ap()
    g2 = nc.dram_tensor("g2", [128, NB], BF, kind="ExternalOutput").ap()
    with tile.TileContext(nc) as tc:
        with tc.tile_pool(name="sb", bufs=5) as sb, \
             tc.tile_pool(name="rs", bufs=2) as rs, \
             tc.tile_pool(name="ps", bufs=4, space="PSUM") as pp, \
             tc.tile_pool(name="cst", bufs=1) as cst:
            cst_t = cst.tile([128, CONST2], BF)
            g2f = cst.tile([128, NB], FT)
            g2_t = cst.tile([128, NB], BF)
            eye = cst_t[:, C2_EYE:C2_EYE + 128]
            first = True
            for (coff, clen, segs) in chunks:
                st = sb.tile([128, maxlen], BF, tag="slot")
                nc.sync.dma_start(out=st[:, :clen],
                                  in_=slots[:, coff:coff + clen])
                if first:
                    nc.vector.tensor_copy(out=cst_t[:], in_=st[:, :CONST2])
                    first = False
                nbt = sum(s[2] for s in segs)
                b0c = segs[0][1]
                res = rs.tile([128, maxnb * H], BF if fast else FT,
                              tag="res")
                fl = rs.tile([128, maxlen // 2], BF, tag="fl")
                pe_ns, dve_ns = 0.0, 700.0   # DVE also runs the chunk tail
                for (cap, b0, nb, soff) in segs:
                    m = nb * H
                    ro = res[:, (b0 - b0c) * H:(b0 - b0c + nb) * H]
                    # greedy makespan balance (measured rates):
                    # PE psum-accumulate vs DVE tree-fold
                    nlev = cap.bit_length() + 2
                    cost_pe = cap * (160.0 + m * 0.40)
                    cost_dve = cap * m * 0.62 + 180.0 * nlev
                    if max(pe_ns + cost_pe, dve_ns) <= max(
                            pe_ns, dve_ns + cost_dve):
                        pe_ns += cost_pe
                        acc = pp.tile([128, 512], FT, space="PSUM",
                                      tag="acc")
                        for wi in range(cap):
                            nc.tensor.matmul(
                                out=acc[:, :m], lhsT=eye,
                                rhs=st[:, soff + wi * m:soff + (wi + 1) * m],
                                start=(wi == 0), stop=(wi == cap - 1))
                        # fast path: relu fused into the PSUM->SBUF copy
                        nc.scalar.activation(
                            out=ro, in_=acc[:, :m],
                            func=(mybir.ActivationFunctionType.Relu if fast
                                  else mybir.ActivationFunctionType.Copy))
                    else:
                        dve_ns += cost_dve
                        v = st[:, soff:soff + nb * H * cap].rearrange(
                            "p (w m) -> p w m", w=cap, m=m)
                        _emit_folds(nc, v, ro.unsqueeze(1), scratch=fl)
                        if fast:
                            nc.vector.tensor_scalar(
                                out=ro, in0=ro, scalar1=0.0, scalar2=None,
                                op0=mybir.AluOpType.max)
                rv = res[:, :nbt * H]
                rview = rv.rearrange("p (b c) -> p b c", b=nbt, c=H)
                if fast:
                    # g2 = dis^2 * sum_c relu(S_c) * w2_c   (valid b1==0)
                    w2b = cst_t[:, C2_W2:C2_W2 + H].unsqueeze(1).to_broadcast(
                        [128, nbt, H])
                    nc.vector.tensor_tensor(out=rview, in0=rview, in1=w2b,
                                            op=MULT)
                    nc.vector.tensor_reduce(
                        out=g2f[:, b0c:b0c + nbt], in_=rview,
                        axis=mybir.AxisListType.X, op=ADD)
                    dis2v = cst_t[:, C2_DIS2:C2_DIS2 + 196].bitcast(FT)
                    nc.vector.tensor_tensor(
                        out=g2_t[:, b0c:b0c + nbt], in0=g2f[:, b0c:b0c + nbt],
                        in1=dis2v[:, b0c:b0c + nbt], op=MULT)
                else:
                    # general: g2 = dis*(relu(dis*S + b1) @ w2)
                    disv = cst_t[:, C2_DIS:C2_DIS + 196].bitcast(FT)
                    disb = disv[:, b0c:b0c + nbt].unsqueeze(2).to_broadcast(
                        [128, nbt, H])
                    nc.vector.tensor_tensor(out=rview, in0=rview, in1=disb,
                                            op=MULT)
                    b1v = cst_t[:, C2_B1:C2_B1 + 32].bitcast(FT)
                    b1b = b1v.unsqueeze(1).to_broadcast([128, nbt, H])
                    nc.vector.tensor_tensor(out=rview, in0=rview, in1=b1b,
                                            op=ADD)
                    nc.vector.tensor_scalar(
                        out=rv, in0=rv, scalar1=0.0, scalar2=None,
                        op0=mybir.AluOpType.max)
                    w2b = cst_t[:, C2_W2:C2_W2 + H].unsqueeze(1).to_broadcast(
                        [128, nbt, H])
                    nc.vector.tensor_tensor(out=rview, in0=rview, in1=w2b,
                                            op=MULT)
                    nc.vector.tensor_reduce(
                        out=g2f[:, b0c:b0c + nbt], in_=rview,
                        axis=mybir.AxisListType.X, op=ADD)
                    nc.vector.tensor_tensor(
                        out=g2_t[:, b0c:b0c + nbt], in0=g2f[:, b0c:b0c + nbt],
                        in1=disv[:, b0c:b0c + nbt], op=MULT)
            nc.sync.dma_start(out=g2[:], in_=g2_t[:])
    nc.compile()
    return nc


# ----------------------------------------------------------------------
# L3: conv2 segment reduce + bias
# ----------------------------------------------------------------------
def _build_l3(caps):
    total, chunks = _plan_chunks(caps, 1, CONST3, CHUNK3)
    maxlen = max(cl for _, cl, _ in chunks)
    nc = bacc.Bacc("TRN2", target_bir_lowering=False, debug=False,
                   num_devices=NC)
    slots = nc.dram_tensor("slots", [128, total], BF,
                           kind="ExternalInput").ap()
    out = nc.dram_tensor("out", [128, NB], FT, kind="ExternalOutput").ap()
    with tile.TileContext(nc) as tc:
        with tc.tile_pool(name="sb", bufs=3) as sb, \
             tc.tile_pool(name="cst", bufs=1) as cst:
            cst_t = cst.tile([128, CONST3], BF)
            resf = cst.tile([128, NB], FT)
            out_t = cst.tile([128, NB], FT)
            first = True
            for (coff, clen, segs) in chunks:
                st = sb.tile([128, maxlen], BF, tag="slot")
                nc.sync.dma_start(out=st[:, :clen],
                                  in_=slots[:, coff:coff + clen])
                if first:
                    nc.vector.tensor_copy(out=cst_t[:], in_=st[:, :CONST3])
                    first = False
                for (cap, b0, nb, soff) in segs:
                    v = st[:, soff:soff + nb * cap].rearrange(
                        "p (b w) -> p b w", b=nb, w=cap)
                    nc.vector.tensor_reduce(out=resf[:, b0:b0 + nb], in_=v,
                                            axis=mybir.AxisListType.X, op=ADD)
            disv = cst_t[:, C3_DIS:C3_DIS + 196].bitcast(FT)
            nc.vector.tensor_tensor(out=out_t[:], in0=resf[:], in1=disv[:],
                                    op=MULT)
            b2v = cst_t[:, C3_B2:C3_B2 + 2].bitcast(FT)
            nc.vector.tensor_scalar(out=out_t[:], in0=out_t[:],
                                    scalar1=b2v, scalar2=None, op0=ADD)
            nc.sync.dma_start(out=out[:], in_=out_t[:])
    nc.compile()
    return nc


def _run(nc, in_maps, label):
    trace = os.environ.get("GCN_TRACE", "0") == "1"
    res = bass_utils.run_bass_kernel_spmd(nc, in_maps,
                                          core_ids=list(range(NC)),
                                          trace=trace)
    if res.exec_time_ns is not None:
        last_exec_ns[label] = res.exec_time_ns
    return res.results


def kernel(x, edge_index, W1, b1, W2, b2):
    import ml_dtypes
    BFNP = ml_dtypes.bfloat16
    x = np.asarray(x, np.float32)
    edge_index = np.asarray(edge_index, np.int32)
    W1 = np.asarray(W1, np.float32)
    b1 = np.asarray(b1, np.float32)
    W2 = np.asarray(W2, np.float32)
    b2 = np.asarray(b2, np.float32)

    # ---- host routing (integer index work only) ----
    loop = np.arange(N, dtype=np.int64)
    src = np.concatenate([edge_index[0].astype(np.int64), loop])
    dst = np.concatenate([edge_index[1].astype(np.int64), loop])
    deg = np.bincount(dst, minlength=N).astype(np.int64)
    order = np.argsort(dst, kind="stable")
    src_s, dst_s = src[order], dst[order]
    core_start = np.searchsorted(dst_s, np.arange(0, N + 1, SH))

    # per-core degree-sorted row assignment + per-block slot caps
    pi = []           # pi[c][r] = global node id at row r (-1 = pad)
    caps_core = np.zeros((NC, NB), np.int64)
    for c in range(NC):
        d_loc = np.zeros(SHP, np.int64)
        d_loc[:SH] = deg[c * SH:(c + 1) * SH]
        ids = np.full(SHP, -1, np.int64)
        ids[:SH] = np.arange(c * SH, (c + 1) * SH)
        o = np.argsort(d_loc, kind="stable")
        pi.append(ids[o])
        dsorted = d_loc[o]
        caps_core[c] = np.maximum(
            2, ((dsorted.reshape(NB, 128).max(axis=1) + 1) // 2) * 2)
    caps = tuple(int(v) for v in caps_core.max(axis=0))
    offs1 = CONST3 + np.concatenate(
        [[0], np.cumsum(caps)]).astype(np.int64)
    # per-block column base / w-stride of the w-major L2 segment layout
    COLS2, chunks2 = _plan_chunks(caps, H, CONST2, CHUNK2)
    colbase2 = np.zeros(NB, np.int64)
    wstride2 = np.zeros(NB, np.int64)
    for (coff, clen, segs) in chunks2:
        for (cap, b0, nb, soff) in segs:
            for i in range(nb):
                colbase2[b0 + i] = coff + soff + i * H
                wstride2[b0 + i] = nb * H
    COLS3 = int(offs1[-1])

    dis_full = np.where(deg > 0, 1.0 / np.sqrt(deg.astype(np.float64)),
                        0.0).astype(np.float32)
    dis2_full = np.where(deg > 0, 1.0 / deg.astype(np.float64),
                         0.0).astype(np.float32)

    # ---- L1: g = dis * (x @ W1) on device ----
    l1 = _cached.get("l1") or _cached.setdefault("l1", _build_l1())
    in_maps1 = []
    for c in range(NC):
        xs = np.zeros((SHP, F), np.float32)
        xs[:SH] = x[c * SH:(c + 1) * SH]
        dis_sh = np.zeros(SHP, np.float32)
        dis_sh[:SH] = dis_full[c * SH:(c + 1) * SH]
        xin = np.zeros((128, 16 + SLC + SHP), BFNP)
        xin[:, 0:16] = W1.astype(BFNP)
        xin[:, 16:16 + SLC] = np.repeat(
            dis_sh.reshape(4, SLC), 32, axis=0).astype(BFNP)
        xin[:, 16 + SLC:] = np.ascontiguousarray(xs.T).astype(BFNP)
        in_maps1.append({"xin": xin})
    res1 = _run(l1, in_maps1, "l1")
    g_full = np.zeros((N, H), BFNP)
    for c in range(NC):
        gc = res1[c]["g"].reshape(4, 32, SLC)[:, :16, :].transpose(
            0, 2, 1).reshape(SHP, H)
        g_full[c * SH:(c + 1) * SH] = gc[:SH]

    # ---- per-core slot coordinates (host, reused for L2/L3) ----
    coords = []       # (p_e, col0_2, cap_e, col_3, srcs_e)
    dis_dev = []
    dis2_dev = []
    for c in range(NC):
        rows = pi[c]
        r = np.arange(SHP)
        valid = rows >= 0
        safe = np.where(valid, rows, 0)
        dis_t = np.zeros((128, NB), np.float32)
        dis_t[r % 128, r // 128] = np.where(valid, dis_full[safe], 0.0)
        dis_dev.append(dis_t)
        dis2_t = np.zeros((128, NB), np.float32)
        dis2_t[r % 128, r // 128] = np.where(valid, dis2_full[safe], 0.0)
        dis2_dev.append(dis2_t)
        rr = r[valid]
        nodes_r = rows[valid]
        st = core_start[c] + np.searchsorted(
            dst_s[core_start[c]:core_start[c + 1]], nodes_r)
        cnt = deg[nodes_r]
        rep_r = np.repeat(rr, cnt)
        w_e = np.arange(len(rep_r)) - np.repeat(np.cumsum(cnt) - cnt, cnt)
        srcs_e = src_s[np.repeat(st, cnt) + w_e]
        b_e = rep_r // 128
        p_e = rep_r % 128
        col0_2 = colbase2[b_e] + w_e * wstride2[b_e]
        col_3 = offs1[b_e] + w_e
        coords.append((p_e, col0_2, col_3, srcs_e))

    # ---- L2: conv1 reduce + relu + W2 on device ----
    fast = bool(np.all(b1 == 0.0))
    key2 = ("l2", caps, fast)
    l2 = _cached.get(key2) or _cached.setdefault(key2,
                                                 _build_l2(caps, fast))
    w2_rep = np.tile(W2[:, 0][None, :], (128, 1)).astype(BFNP)
    b1_rep = np.tile(b1[None, :], (128, 1)).astype(np.float32)
    eye_rep = np.eye(128, dtype=BFNP)
    in_maps2 = []
    for c in range(NC):
        p_e, col0_2, _, srcs_e = coords[c]
        sl = np.zeros((128, COLS2), BFNP)
        sl[:, C2_DIS2:C2_DIS2 + 196] = dis2_dev[c].view(BFNP)
        sl[:, C2_DIS:C2_DIS + 196] = dis_dev[c].view(BFNP)
        sl[:, C2_B1:C2_B1 + 32] = b1_rep.view(BFNP)
        sl[:, C2_W2:C2_W2 + H] = w2_rep
        sl[:, C2_EYE:C2_EYE + 128] = eye_rep
        gv = g_full[srcs_e]          # [E_c, H] bf16
        for ch in range(H):
            sl[p_e, col0_2 + ch] = gv[:, ch]
        in_maps2.append({"slots": sl})
    res2 = _run(l2, in_maps2, "l2")
    g2_full = np.zeros(N, BFNP)
    for c in range(NC):
        g2c = res2[c]["g2"]
        rows = pi[c]
        r = np.arange(SHP)
        valid = rows >= 0
        g2_full[rows[valid]] = g2c[(r % 128)[valid], (r // 128)[valid]]

    # ---- L3: conv2 reduce on device ----
    key3 = ("l3", caps)
    l3 = _cached.get(key3) or _cached.setdefault(key3, _build_l3(caps))
    b2_rep = np.full((128, 1), float(b2[0]), np.float32)
    in_maps3 = []
    for c in range(NC):
        p_e, _, col_3, srcs_e = coords[c]
        sl = np.zeros((128, COLS3), BFNP)
        sl[:, C3_DIS:C3_DIS + 196] = dis_dev[c].view(BFNP)
        sl[:, C3_B2:C3_B2 + 2] = b2_rep.view(BFNP)
        sl[p_e, col_3] = g2_full[srcs_e]
        in_maps3.append({"slots": sl})
    res3 = _run(l3, in_maps3, "l3")
    out = np.zeros((N, 1), np.float32)
    for c in range(NC):
        oc = res3[c]["out"]
        rows = pi[c]
        r = np.arange(SHP)
        valid = rows >= 0
        out[rows[valid], 0] = oc[(r % 128)[valid], (r // 128)[valid]]
    return out
